# revision 1
# baseline (speedup 1.0000x reference)
"""Int8 AG-GEMM (x @ weight.T with per-row/per-col dequant + bias) on 8 TRN2
NeuronCores.

Strategy: data-parallel over M (rows of x). Core c owns rows
[c*512, (c+1)*512). All inputs are fed fully prepared from the host in the
exact SBUF tile layout, so every DMA source is contiguous per partition
(descriptor generation is then negligible):
  - xt   [XC, 128, K/(128*XC), M_C] int8 : transposed x shard, chunked over K
  - wt   [N/NB, WQ, 128, K/(128*WQ), NB] int8 : transposed weight, tiled
         (replicated to every core)
  - isr  [128, M_C] f32 : input_scale shard replicated across partitions
  - wsr  [128, N/128] f32 : weight_scale, partition-major
  - br   [128, N/128] f32 : bias, partition-major
Each core computes outT = [N, M_C] bf16 (the transposed output shard):
  psum[n-tile 128, M_C] = sum_k wt_tile[k, n].T @ xt_tile[k]   (fp32, exact)
  out = (psum * isr) * ws[n][:,1] + bias[n][:,1] -> bf16  (two DVE ops)
The host transposes each core's outT back and stitches the full [M, N].

The int8 GEMM is exact: int8 values are exact in bf16, products are exact in
the PE's fp32 accumulator, and partial sums stay far below 2^24.

DMA queues: the int8 x/weight streams ride the gpsimd SWDGE queue, which
casts int8->bf16 in flight (halves HBM traffic); block-0 weight quarters
are interleaved with the x chunks in first-use order so the PE starts
~17us in. Scales ride the scalar HWDGE queue; output stores ride sync.
"""

import numpy as np

M_FULL, K_FULL, N_FULL = 4096, 8192, 8192
N_CORES = 8
N_XCHUNK = 8    # x load split (SWDGE granules)
N_WQUART = 4    # weight block k-split (SWDGE granules)
N_PER_BLK = 256


def build_nc(K, N, M_C, n_per_blk=N_PER_BLK):
    """Build the SPMD kernel graph for per-core problem [K, N] x [K, M_C]."""
    import concourse.mybir as mybir
    import concourse.tile as tile
    from concourse import bacc

    bf16 = mybir.dt.bfloat16
    f32 = mybir.dt.float32

    kt = K // 128          # k-tiles
    nt = N // 128          # n-tiles (output partition tiles)
    nblk = N // n_per_blk  # weight streaming blocks
    jt = n_per_blk // 128  # n-tiles per block
    xc_n = min(N_XCHUNK, kt)
    kc = kt // xc_n        # k-tiles per x chunk
    wq_n = min(N_WQUART, kt)
    kq = kt // wq_n        # k-tiles per weight quarter

    i8 = mybir.dt.int8

    nc = bacc.Bacc("TRN2", target_bir_lowering=False, debug=False,
                   num_devices=N_CORES)
    xt = nc.dram_tensor("xt", [xc_n, 128, kc, M_C], i8, kind="ExternalInput")
    wt = nc.dram_tensor("wt", [nblk, wq_n, 128, kq, n_per_blk], i8,
                        kind="ExternalInput")
    # duplicate copies of the k=0 tiles: tiny first DMAs so the first
    # matmul fires before the bulk granules finish streaming
    xk0 = nc.dram_tensor("xk0", [128, M_C], i8, kind="ExternalInput")
    wk0 = nc.dram_tensor("wk0", [128, n_per_blk], i8, kind="ExternalInput")
    isr = nc.dram_tensor("isr", [128, M_C], f32, kind="ExternalInput")
    wsr = nc.dram_tensor("wsr", [128, nt], f32, kind="ExternalInput")
    br = nc.dram_tensor("br", [128, nt], f32, kind="ExternalInput")
    outt = nc.dram_tensor("outt", [N, M_C], bf16, kind="ExternalOutput")

    with tile.TileContext(nc) as tc:
        with (
            tc.tile_pool(name="const", bufs=1) as cpool,
            tc.tile_pool(name="wstream", bufs=3 * wq_n) as wpool,
            tc.tile_pool(name="psum", bufs=4, space="PSUM") as ppool,
            tc.tile_pool(name="t1", bufs=4) as t1pool,
            tc.tile_pool(name="osb", bufs=4) as opool,
        ):
            xch = [cpool.tile([128, kc, M_C], bf16, tag=f"xsb{c}",
                              name=f"xsb{c}")
                   for c in range(xc_n)]

            def dma_x(c):
                nc.gpsimd.dma_start(xch[c][:], xt.ap()[c])

            # Interleave x-chunk loads with block-0 weight quarters on the
            # SWDGE queue so the first psum group's deps land first: the
            # group's k-range for weight quarter q needs x chunks 2q, 2q+1.
            x_per_q = max(1, xc_n // wq_n)
            xk0_sb = cpool.tile([128, M_C], bf16)
            nc.gpsimd.dma_start(xk0_sb[:], xk0.ap())
            wk0_sb = cpool.tile([128, n_per_blk], bf16)
            nc.gpsimd.dma_start(wk0_sb[:], wk0.ap())
            dma_x(0)
            x_issued = 1
            isr_sb = cpool.tile([128, M_C], f32)
            nc.scalar.dma_start(isr_sb[:], isr.ap())
            ws_sb = cpool.tile([128, nt], f32)
            nc.scalar.dma_start(ws_sb[:], wsr.ap())
            b_sb = cpool.tile([128, nt], f32)
            nc.scalar.dma_start(b_sb[:], br.ap())

            for s in range(nblk):
                wqs = []
                for q in range(wq_n):
                    wq = wpool.tile([128, kq, n_per_blk], bf16, tag="wq")
                    nc.gpsimd.dma_start(wq[:], wt.ap()[s, q])
                    wqs.append(wq)
                    if s == 0:
                        for _ in range(x_per_q):
                            if x_issued < xc_n:
                                dma_x(x_issued)
                                x_issued += 1
                while x_issued < xc_n:
                    dma_x(x_issued)
                    x_issued += 1
                for j in range(jt):
                    n = s * jt + j
                    ps = ppool.tile([128, M_C], f32)
                    for k in range(kt):
                        if s == 0 and k == 0:
                            wsrc = wk0_sb[:, j * 128:(j + 1) * 128]
                            xsrc = xk0_sb[:]
                        else:
                            wsrc = wqs[k // kq][:, k % kq,
                                               j * 128:(j + 1) * 128]
                            xsrc = xch[k // kc][:, k % kc, :]
                        nc.tensor.matmul(
                            ps[:], wsrc, xsrc,
                            start=(k == 0),
                            stop=(k == kt - 1),
                        )
                    t1 = t1pool.tile([128, M_C], f32)
                    nc.vector.tensor_tensor(
                        t1[:], ps[:], isr_sb[:], mybir.AluOpType.mult
                    )
                    ob = opool.tile([128, M_C], bf16)
                    nc.vector.tensor_scalar(
                        ob[:], t1[:],
                        ws_sb[:, n:n + 1], b_sb[:, n:n + 1],
                        mybir.AluOpType.mult, mybir.AluOpType.add,
                    )
                    nc.sync.dma_start(outt.ap()[n * 128:(n + 1) * 128, :], ob[:])

    nc.compile()
    return nc


def prep_in_maps(x, weight, bias, input_scale, weight_scale, n_cores=N_CORES,
                 n_per_blk=N_PER_BLK):
    """Host-side shard + SBUF-layout prep. Returns (in_maps, M_C)."""
    import ml_dtypes

    bf16 = ml_dtypes.bfloat16
    M, K = x.shape
    N = weight.shape[0]
    M_C = M // n_cores
    kt = K // 128
    xc_n = min(N_XCHUNK, kt)
    kc = kt // xc_n
    wq_n = min(N_WQUART, kt)
    kq = kt // wq_n
    nblk = N // n_per_blk

    xt_full = np.ascontiguousarray(x.T).astype(np.int8)  # [K, M]
    wt = np.ascontiguousarray(weight.T).astype(np.int8)  # [K, N]
    # [K, N] -> [nblk, wq_n, 128, kq, n_per_blk];  K = wq_n*kq*128
    wt_t = np.ascontiguousarray(
        wt.reshape(wq_n, kq, 128, nblk, n_per_blk).transpose(3, 0, 2, 1, 4))
    wsr = np.ascontiguousarray(
        weight_scale.astype(np.float32).reshape(N // 128, 128).T)
    br = np.ascontiguousarray(bias.astype(np.float32).reshape(N // 128, 128).T)

    in_maps = []
    for c in range(n_cores):
        sl = slice(c * M_C, (c + 1) * M_C)
        # [K, M_C] -> [xc_n, 128, kc, M_C]
        xt_c = np.ascontiguousarray(
            xt_full[:, sl].reshape(xc_n, kc, 128, M_C).transpose(0, 2, 1, 3))
        in_maps.append({
            "xt": xt_c,
            "wt": wt_t,
            "xk0": np.ascontiguousarray(xt_c[0, :, 0, :]),
            "wk0": np.ascontiguousarray(wt_t[0, 0, :, 0, :]),
            "isr": np.ascontiguousarray(
                np.broadcast_to(input_scale[sl].astype(np.float32)[None, :],
                                (128, M_C))),
            "wsr": wsr,
            "br": br,
        })
    return in_maps, M_C


def run(x, weight, bias, input_scale, weight_scale, trace=False):
    """Run the SPMD kernel; returns (out [M, N] bf16, BassKernelResults)."""
    from concourse.bass_utils import run_bass_kernel_spmd

    M, K = x.shape
    N = weight.shape[0]
    in_maps, M_C = prep_in_maps(x, weight, bias, input_scale, weight_scale)
    nc = build_nc(K, N, M_C)
    res = run_bass_kernel_spmd(nc, in_maps, list(range(N_CORES)), trace=trace)

    import ml_dtypes
    out = np.empty((M, N), dtype=ml_dtypes.bfloat16)
    for c in range(N_CORES):
        out[c * M_C:(c + 1) * M_C, :] = res.results[c]["outt"].T
    return out, res


def kernel(x, weight, bias, input_scale, weight_scale):
    x, weight, bias, input_scale, weight_scale = (
        np.asarray(a) for a in (x, weight, bias, input_scale, weight_scale))
    out, _ = run(x, weight, bias, input_scale, weight_scale, trace=False)
    return out

